# revision 28
# baseline (speedup 1.0000x reference)
"""Causal self-attention (B=2, T=2048, C=1024, H=16) on 8 trn2 NeuronCores.

Sharding: data-parallel over B (2) x tensor-parallel over head groups (4
groups of 4 heads).  core c -> batch c//4, head group c%4.  Each core
computes its 4 heads' qkv projection, attention, and the partial c_proj
contribution; the host sums the 4 tensor-parallel partials per batch
(the "all-reduce") and adds b_proj.

All matmul operands are bf16 (fp32 PSUM accumulation).  fp32-mode
(F32R) matmuls trip the PE power throttle to K=4/8 for most of the
kernel; bf16 runs the array at full clock.  Phases are software-
pipelined over 512-wide t-blocks j: causality means attention for
q-block j only needs k/v through block j, so qkv(j+1) | attn(j) |
normalize+c_proj(j-1) overlap across engines.
"""

import sys
from contextlib import ExitStack

for _p in ("/opt/trn_rl_repo",):
    if _p not in sys.path:
        sys.path.insert(0, _p)

import ml_dtypes
import numpy as np

import concourse.bass as bass
import concourse.tile as tile
from concourse import mybir
from concourse.bass_utils import run_bass_kernel_spmd

F32 = mybir.dt.float32
F32R = mybir.dt.float32r
BF16 = mybir.dt.bfloat16
EXP = mybir.ActivationFunctionType.Exp

B, T, C = 2, 2048, 1024
H, D = 16, 64          # total heads, head dim
HL = 4                 # heads per core (local)
N_CORES = 8
QB = 512               # q block width (columns of S_T)
NTT = T // 128         # 16 t-tiles
NTB = T // QB          # 4 t-blocks
NC_C = C // 128        # 8 contraction tiles over C


def _merge(a, b):
    for k, v in b.items():
        if a.get(k, -1) < v:
            a[k] = v


def _reduce_matmul_waits(nc):
    """Sound transitive reduction of Matmult sync waits.

    Walrus rejects self-loading matmuls with >1 sync wait (the LDWEIGHTS
    struct has one wait slot).  Tile emits per-proc-minimal waits but does not
    track cross-proc transitivity, so e.g. a matmul recycling a PSUM slot
    waits on both the old writer (PE) and the old reader (ACT) even though the
    reader's wait already implies the writer finished.  We compute guarantee
    vector clocks (sem -> min value) for every sem increment and drop Matmult
    waits that are implied by the instruction's queue dispatch knowledge plus
    its remaining waits.
    """
    import bass_rust
    DMA_OPS = {"InstDMACopy", "InstDMATranspose"}
    dispatch = {}    # queue -> clock known at sequencer dispatch point
    done_prev = {}   # queue -> completion clock of previous engine inst
    sem_cum = {}     # sem -> cumulative inc
    sem_hist = {}    # sem -> list[(cum, prefix-merged clock)]
    n_dropped = 0

    def clock_at(sem, v):
        for cum, snap in sem_hist.get(sem, ()):
            if cum >= v:
                return snap
        return {}

    insts = [ins for bb in nc.main_func.blocks for ins in bb.instructions]
    for ins in insts:
        si = ins.sync_info
        q = str(getattr(ins, "engine", "?"))
        opc = type(ins).__name__
        dq = dispatch.setdefault(q, {})
        waits = list(si.on_wait) if si is not None else []
        wclocks = []
        for w in waits:
            wc = dict(clock_at(w.ant_name, w.wait_value))
            if wc.get(w.ant_name, -1) < w.wait_value:
                wc[w.ant_name] = w.wait_value
            wclocks.append(wc)

        if len(waits) > 1:
            # For serially-executing engines (DVE drains its pipe per op; ACT
            # and GpSimd likewise retire in order), the previous same-queue
            # instruction has fully completed by the time this one executes,
            # so its completion clock joins the implication base.  PE overlaps
            # matmul drains, and DMA lanes are async, so they only get
            # sequencer dispatch knowledge.
            serial = opc not in DMA_OPS and not q.endswith("PE")
            keep = set(range(len(waits)))
            order = sorted(
                range(len(waits)),
                key=lambda k: 0 if not waits[k].ant_name.startswith("DMA") else 1,
            )
            for k in order:
                if len(keep) <= 1:
                    break
                base = dict(dq)
                if serial:
                    _merge(base, done_prev.get(q, {}))
                for k2 in keep:
                    if k2 != k:
                        _merge(base, wclocks[k2])
                w = waits[k]
                if base.get(w.ant_name, -1) >= w.wait_value:
                    keep.discard(k)
            if len(keep) < len(waits):
                n_dropped += len(waits) - len(keep)
                ins.sync_info = bass_rust.SyncInfo(
                    on_wait=[waits[k] for k in sorted(keep)],
                    on_update=list(si.on_update),
                )

        for wc in wclocks:
            _merge(dq, wc)

        comp = dict(dq)
        if opc not in DMA_OPS:
            _merge(comp, done_prev.get(q, {}))

        ups = list(si.on_update) if si is not None else []
        for u in ups:
            if u.update_mode != "sem-inc":
                continue
            cum = sem_cum.get(u.ant_name, 0) + u.update_value
            sem_cum[u.ant_name] = cum
            hist = sem_hist.setdefault(u.ant_name, [])
            snap = dict(hist[-1][1]) if hist else {}
            _merge(snap, comp)
            snap[u.ant_name] = cum
            hist.append((cum, snap))
        if opc not in DMA_OPS:
            comp2 = dict(comp)
            for u in ups:
                if u.update_mode == "sem-inc":
                    comp2[u.ant_name] = max(
                        comp2.get(u.ant_name, 0), sem_cum[u.ant_name])
            done_prev[q] = comp2

    bad = [
        (ins.name, [(w.ant_name, w.wait_value) for w in ins.sync_info.on_wait])
        for ins in insts
        if type(ins).__name__ == "InstMatmult"
        and ins.sync_info is not None and len(ins.sync_info.on_wait) > 1
    ]
    if bad:
        raise RuntimeError(f"{len(bad)} matmuls still have >1 wait: {bad[:8]}")

    # This walrus accepts at most ONE sync wait per instruction struct.
    # Matmuls are handled above; for everything else, hoist the extra waits
    # onto standalone single-wait Drain carriers on the same queue (the
    # sequencer executes them in order, so the semantics are unchanged).
    wid = 0
    for bb in nc.main_func.blocks:
        out_list = []
        changed = False
        for ins in bb.instructions:
            si = ins.sync_info
            if (si is not None and len(si.on_wait) > 1
                    and type(ins).__name__ != "InstMatmult"):
                waits = list(si.on_wait)
                for w in waits[:-1]:
                    d = mybir.InstDrain(name=f"WSPLIT-{wid}", ins=[], outs=[])
                    wid += 1
                    d.engine = ins.engine
                    d.sync_info = bass_rust.SyncInfo(on_wait=[w], on_update=[])
                    try:
                        nc.register_instruction(d)
                    except Exception:
                        pass
                    out_list.append(d)
                ins.sync_info = bass_rust.SyncInfo(
                    on_wait=[waits[-1]], on_update=list(si.on_update))
                changed = True
            out_list.append(ins)
        if changed:
            bb.instructions = out_list

    # This neuronxcc's walrus rejects the raw-ISA EVENT_SEMAPHORE_RANGE_CLEAR
    # Tile emits as end-of-program semaphore hygiene ("ISA wrong length").
    # It has no sync side effects and only matters for back-to-back reuse of
    # the semaphore window inside one program, so drop it.
    for bb in nc.main_func.blocks:
        kept = [i for i in bb.instructions
                if not (type(i).__name__ == "InstISA"
                        and getattr(i, "op_name", "") ==
                        "EVENT_SEMAPHORE_RANGE_CLEAR")]
        if len(kept) != len(bb.instructions):
            bb.instructions = kept


def _build_nc() -> bass.Bass:
    nc = bass.Bass()

    xt = nc.declare_dram_parameter("xt", [C, T], BF16, False)
    wqk = nc.declare_dram_parameter("wqk", [C, 512], BF16, False)
    bqk = nc.declare_dram_parameter("bqk", [1, 512], BF16, False)
    wv = nc.declare_dram_parameter("wv", [C, 256], BF16, False)
    bv = nc.declare_dram_parameter("bv", [1, 256], BF16, False)
    wp2 = nc.declare_dram_parameter("wp2", [128, 2, C], BF16, False)
    tri = nc.declare_dram_parameter("tri", [128, 128], F32, False)
    out = nc.declare_dram_parameter("out", [T, C], F32, True)

    with tile.TileContext(nc) as tc, ExitStack() as ctx:
        consts = ctx.enter_context(tc.tile_pool(name="consts", bufs=1))
        wpool = ctx.enter_context(tc.tile_pool(name="wpool", bufs=1))
        big = ctx.enter_context(tc.tile_pool(name="big", bufs=1))
        xtp = ctx.enter_context(tc.tile_pool(name="xtp", bufs=2))
        ppool = ctx.enter_context(tc.tile_pool(name="ppool", bufs=4))
        opool = ctx.enter_context(tc.tile_pool(name="opool", bufs=2))
        small = ctx.enter_context(tc.tile_pool(name="small", bufs=2))
        # PSUM budget (8 banks): "ps" holds everything but psy in two
        # 2-bank slots (the S pairs set the slot size; qkv/c_proj/psb tiles
        # use the slot's first bank); psy0/psy1 double-buffered.
        psp = ctx.enter_context(tc.tile_pool(name="psp", bufs=2, space="PSUM"))
        psyp = ctx.enter_context(tc.tile_pool(name="psyp", bufs=2, space="PSUM"))

        # ---- constants + first-needed weights (DMA order matters: the
        # ring drains in issue order, so qk weights + x(0) come first and
        # the first matmul doesn't wait on the v/c_proj weights) ----
        tri_sb = consts.tile([128, 128], F32)   # additive: 0 if kk<=cc else -1e30
        nc.sync.dma_start(out=tri_sb, in_=tri[:])
        ones = consts.tile([1, 512], BF16)
        warm = consts.tile([1, 8], F32)

        bqk_sb = consts.tile([1, 512], BF16)
        nc.sync.dma_start(out=bqk_sb, in_=bqk[:])
        wqk_sb = wpool.tile([128, NC_C, 512], BF16)
        nc.sync.dma_start(out=wqk_sb, in_=wqk[:].rearrange("(c p) n -> p c n", p=128))

        # ---- persistent activations ----
        qkT = big.tile([128, 4, T], BF16)    # [qk ct, t]: rows = [q01|q23|k01|k23]*64
        v_sb = big.tile([128, NTT, HL, 65], BF16)   # v natural + ones column
        # normalized y, head-pair packed: ypar[g] rows hp*64+d = head 2g+hp
        ypars = [
            big.tile([128, T], BF16, tag=f"yp{g}", name=f"yp{g}") for g in range(2)
        ]
        sums2 = big.tile([1, 16, 512], F32)   # raw denominators, col (4j+h)
        sums2r = big.tile([1, 16, 512], F32R)  # 1/denominators, DVE-written
        ones_f = consts.tile([1, 64], F32R)

        # all constant memsets on DVE (single-proc tick, observed once by PE)
        nc.vector.memset(v_sb[:, :, :, 64:65], 1.0)
        nc.vector.memset(ones, 1.0)
        nc.vector.memset(ones_f.bitcast(F32), 1.0)
        nc.vector.memset(warm, 0.0)
        nc.scalar.activation(out=warm, in_=warm, func=EXP)  # pre-load exp table

        # Self-loading matmuls support a single sync wait, so every real
        # matmul must have at most one un-observed dependency.  Funnel all
        # input-DMA completions through DVE copies, then observe DVE once on
        # the PE via a tiny "touch" matmul.  Touches write a psp slot and
        # get a DVE reader so their slot release is a DVE-only event.
        def touch(src_ap):
            # A standalone bf16 LDWEIGHTS makes the PE observe the DVE
            # funnel without a PSUM slot (the next matmul reloads weights).
            nc.tensor.ldweights(weights=src_ap)

        def funnel_touch(aps, k):
            scr = consts.tile([1, 8], BF16, tag=f"scr{k}", name=f"scr{k}")
            for i, t_ap in enumerate(aps):
                nc.vector.tensor_copy(out=scr[:, i:i + 1], in_=t_ap)
            scr2 = consts.tile([1, 1], BF16, tag=f"scr2{k}", name=f"scr2{k}")
            nc.vector.tensor_copy(out=scr2, in_=scr[:, 0:1])
            touch(scr2)

        funnel_touch([wqk_sb[0:1, 0, 0:1], tri_sb[0:1, 0:1], bqk_sb[:, 0:1]], 0)

        # ---------------- phase pieces ----------------

        def load_x(tb):
            ts = slice(tb * QB, (tb + 1) * QB)
            xt_tb = xtp.tile([128, NC_C, QB], BF16, tag="xt", name="xt_tb")
            nc.sync.dma_start(
                out=xt_tb, in_=xt[:, ts].rearrange("(c p) n -> p c n", p=128)
            )
            xt_t = xtp.tile([1, 1], BF16, tag="xt_t", name="xt_t")
            nc.vector.tensor_copy(out=xt_t, in_=xt_tb[0:1, 0, 0:1])
            touch(xt_t)
            return xt_tb

        def qkv_qk(tb, xt_tb):
            """q,k for t-block tb into qkT (ACT copies)."""
            ts = slice(tb * QB, (tb + 1) * QB)
            for ct in range(4):
                ps = psp.tile([128, 512], F32, tag="ps", name="ps")
                for c in range(NC_C):
                    nc.tensor.matmul(
                        out=ps,
                        lhsT=wqk_sb[:, c, ct * 128:(ct + 1) * 128],
                        rhs=xt_tb[:, c, :],
                        start=(c == 0),
                        stop=False,
                    )
                nc.tensor.matmul(  # + bias (outer product with ones row)
                    out=ps,
                    lhsT=bqk_sb[:, ct * 128:(ct + 1) * 128],
                    rhs=ones[:, 0:QB],
                    start=False,
                    stop=True,
                )
                nc.scalar.copy(out=qkT[:, ct, ts], in_=ps)

        def qkv_v(tb, xt_tb):
            """v for the 4 t-tiles of block tb into v_sb (DVE copies)."""
            for t4 in range(4):
                tt = tb * 4 + t4
                psv = psp.tile([128, 256], F32, tag="ps", name="psv")
                for c in range(NC_C):
                    nc.tensor.matmul(
                        out=psv,
                        lhsT=xt_tb[:, c, t4 * 128:(t4 + 1) * 128],
                        rhs=wv_sb[:, c, :],
                        start=(c == 0),
                        stop=False,
                    )
                nc.tensor.matmul(
                    out=psv,
                    lhsT=ones[:, 0:128],
                    rhs=bv_sb[:],
                    start=False,
                    stop=True,
                )
                nc.vector.tensor_copy(
                    out=v_sb[:, tt, :, 0:64],
                    in_=psv[:].rearrange("p (h d) -> p h d", h=HL),
                )

        def attn(j):
            """Attention for q-block j, all 4 heads.  The two heads of a
            pair write adjacent PSUM banks of one [128,2,512] tile so a
            single ACT exp covers both; S/exp run one i-tile ahead of AV
            so the exp hides under PE matmuls."""
            qs = slice(j * QB, (j + 1) * QB)
            nk = 4 * j + 4
            for g in range(2):
                psy = {}
                for hp in range(2):
                    psy[hp] = psyp.tile(
                        [65, 512], F32, tag=f"psy{hp}", name=f"psy{hp}"
                    )
                pts = None  # exp'd pair tile awaiting AV

                def s_exp(i):
                    o = max(0, 128 * i - 512 * j)
                    s2 = psp.tile([128, 2, 512], F32, tag="ps", name="s2")
                    for hp in range(2):
                        h = 2 * g + hp
                        ct_q, ct_k, base = h // 2, 2 + h // 2, (h % 2) * 64
                        rows = slice(base, base + 64)
                        nc.tensor.matmul(
                            out=s2[:, hp, o:512],
                            lhsT=qkT[rows, ct_k, i * 128:(i + 1) * 128],
                            rhs=qkT[rows, ct_q, j * QB + o:(j + 1) * QB],
                            start=True,
                            stop=True,
                            skip_group_check=True,
                        )
                        if i >= 4 * j:  # diagonal block: -inf above diagonal
                            nc.vector.tensor_add(
                                out=s2[:, hp, o:o + 128],
                                in0=s2[:, hp, o:o + 128],
                                in1=tri_sb,
                            )
                    p_t = ppool.tile([128, 2, 512], BF16, tag="p", name="p_t")
                    nc.scalar.activation(
                        out=p_t[:, :, o:512], in_=s2[:, :, o:512], func=EXP,
                        scale=0.125,
                    )
                    return (p_t, o)

                def av(i):
                    p_t, o = pts
                    for hp in range(2):
                        h = 2 * g + hp
                        nc.tensor.matmul(
                            out=psy[hp][:, o:512],
                            lhsT=v_sb[:, i, h, :],
                            rhs=p_t[:, hp, o:512],
                            start=(i == 0),
                            stop=(i == nk - 1),
                        )

                for i in range(nk):
                    nxt = s_exp(i)
                    if i > 0:
                        av(i - 1)
                    pts = nxt
                av(nk - 1)

                for hp in range(2):
                    h = 2 * g + hp
                    nc.vector.tensor_copy(
                        out=sums2[:, 4 * j + h, :],
                        in_=psy[hp][64:65, :],
                    )
                    nc.vector.tensor_copy(
                        out=ypars[g][64 * hp:64 * hp + 64, qs],
                        in_=psy[hp][0:64, :],
                    )
                # sums -> 1/sums in place: exp(-ln s) on ACT (Ln and Exp
                # share the natural_log_exp table set -> no table switch).
                # Emitted per head-pair so g=0's recip runs under g=1's
                # matmuls instead of blocking the next block's exps.  The
                # DVE copy into sums2r funnels the ACT write through DVE so
                # the psb broadcast matmuls end up with a single (DVE) wait.
                s_j = sums2[:, 4 * j + 2 * g:4 * j + 2 * g + 2, :]
                nc.scalar.activation(
                    out=s_j, in_=s_j, func=mybir.ActivationFunctionType.Ln,
                )
                nc.scalar.activation(out=s_j, in_=s_j, func=EXP, scale=-1.0)
                nc.vector.tensor_copy(
                    out=sums2r[:, 4 * j + 2 * g:4 * j + 2 * g + 2, :], in_=s_j,
                )

        def napply(j):
            """Broadcast 1/sums via PE outer product, scale ypar."""
            qs = slice(j * QB, (j + 1) * QB)
            for g in range(2):
                for hp in range(2):
                    h = 2 * g + hp
                    psb = psp.tile([64, 512], F32, tag="ps", name="psb")
                    nc.tensor.matmul(
                        out=psb,
                        lhsT=ones_f,
                        rhs=sums2r[:, 4 * j + h, :],
                        start=True, stop=True,
                    )
                    nc.vector.tensor_mul(
                        out=ypars[g][64 * hp:64 * hp + 64, qs],
                        in0=ypars[g][64 * hp:64 * hp + 64, qs],
                        in1=psb,
                    )

        def cproj(j):
            """c_proj partial for the 4 t-tiles of block j (head pairs give
            K=128), staged through SBUF (DVE) then DMA'd out."""
            for t4 in range(4):
                tt = 4 * j + t4
                t128 = slice(tt * 128, (tt + 1) * 128)
                ot = opool.tile([128, C], F32, tag="ot", name="ot")
                for co in range(2):
                    pso = psp.tile([128, 512], F32, tag="ps", name="pso")
                    for g in range(2):
                        nc.tensor.matmul(
                            out=pso,
                            lhsT=ypars[g][:, t128],
                            rhs=wp_sb[:, g, co * 512:(co + 1) * 512],
                            start=(g == 0),
                            stop=(g == 1),
                        )
                    nc.vector.tensor_copy(out=ot[:, co * 512:(co + 1) * 512], in_=pso)
                nc.sync.dma_start(out=out[t128, :], in_=ot)

        # ---------------- pipelined schedule ----------------
        # Causality: attn(j) needs qkv only through block j, so qkv(j+1)
        # fills the PE while ACT drains block-j copies/exps; the sums
        # reciprocal (ACT) and its PE broadcast are split so each lands
        # where the target engine has slack.
        xts = [load_x(0)]
        qkv_qk(0, xts[0])
        xts.append(load_x(1))
        # v/c_proj weights arrive behind the qk ones; gate the v matmuls
        # on their own touch so the first qk matmuls don't wait for them.
        wv_sb = wpool.tile([128, NC_C, 256], BF16)
        nc.sync.dma_start(out=wv_sb, in_=wv[:].rearrange("(c p) n -> p c n", p=128))
        bv_sb = consts.tile([1, 256], BF16)
        nc.sync.dma_start(out=bv_sb, in_=bv[:])
        funnel_touch([wv_sb[0:1, 0, 0:1], bv_sb[:, 0:1]], 1)
        qkv_v(0, xts[0])
        qkv_qk(1, xts[1])
        qkv_v(1, xts[1])
        wp_sb = wpool.tile([128, 2, C], BF16)
        nc.sync.dma_start(out=wp_sb, in_=wp2[:])
        funnel_touch([wp_sb[0:1, 0, 0:1]], 2)
        attn(0)
        xts.append(load_x(2))
        qkv_qk(2, xts[2])
        qkv_v(2, xts[2])
        napply(0)
        attn(1)
        xts.append(load_x(3))
        qkv_qk(3, xts[3])
        qkv_v(3, xts[3])
        napply(1)
        cproj(0)
        attn(2)
        napply(2)
        cproj(1)
        attn(3)
        cproj(2)
        napply(3)
        cproj(3)

    _reduce_matmul_waits(nc)
    return nc


_NC_CACHE = {}


def _get_nc():
    if "nc" not in _NC_CACHE:
        _NC_CACHE["nc"] = _build_nc()
    return _NC_CACHE["nc"]


def make_in_maps(x, W_attn, b_attn, W_proj, b_proj):
    x = np.asarray(x, dtype=np.float32)
    W_attn = np.asarray(W_attn, dtype=np.float32)
    b_attn = np.asarray(b_attn, dtype=np.float32)
    W_proj = np.asarray(W_proj, dtype=np.float32)
    kk, cc = np.meshgrid(np.arange(128), np.arange(128), indexing="ij")
    tri = np.where(kk <= cc, 0.0, -1e30).astype(np.float32)
    bf = ml_dtypes.bfloat16
    in_maps = []
    for core in range(N_CORES):
        b, hg = core // 4, core % 4
        qc = slice(hg * 256, (hg + 1) * 256)
        kc = slice(C + hg * 256, C + (hg + 1) * 256)
        vc = slice(2 * C + hg * 256, 2 * C + (hg + 1) * 256)
        # W_proj rows for this head group, packed by head pair:
        # wp2[hp*64+d, g, :] = W_proj[hg*256 + (2g+hp)*64 + d, :]
        wp = W_proj[hg * 256:(hg + 1) * 256].reshape(2, 2, 64, C)  # [g,hp,d,C]
        wp2 = np.ascontiguousarray(wp.transpose(1, 2, 0, 3).reshape(128, 2, C))
        in_maps.append({
            "xt": np.ascontiguousarray(x[b].T).astype(bf),
            "wqk": np.ascontiguousarray(
                np.concatenate([W_attn[:, qc], W_attn[:, kc]], axis=1)).astype(bf),
            "bqk": np.concatenate([b_attn[qc], b_attn[kc]]).reshape(1, 512).astype(bf),
            "wv": np.ascontiguousarray(W_attn[:, vc]).astype(bf),
            "bv": b_attn[vc].reshape(1, 256).astype(bf),
            "wp2": wp2.astype(bf),
            "tri": tri,
        })
    return in_maps


def gather(results, b_proj):
    b_proj = np.asarray(b_proj, dtype=np.float32)
    out = np.empty((B, T, C), dtype=np.float32)
    for b in range(B):
        acc = results[4 * b]["out"].astype(np.float32)
        for g in range(1, 4):
            acc = acc + results[4 * b + g]["out"]
        out[b] = acc + b_proj
    return out


def run(x, W_attn, b_attn, W_proj, b_proj, trace=False):
    nc = _get_nc()
    in_maps = make_in_maps(x, W_attn, b_attn, W_proj, b_proj)
    res = run_bass_kernel_spmd(nc, in_maps, list(range(N_CORES)), trace=trace)
    return gather(res.results, b_proj), res


def kernel(x, W_attn, b_attn, W_proj, b_proj):
    out, _ = run(x, W_attn, b_attn, W_proj, b_proj)
    return out


# revision 39
# speedup vs baseline: 1.2440x; 1.2440x over previous
"""Causal self-attention (B=2, T=2048, C=1024, H=16) on 8 trn2 NeuronCores.

Sharding: data-parallel over B (2) x tensor-parallel over head groups (4
groups of 4 heads).  core c -> batch c//4, head group c%4.  Each core
computes its 4 heads' qkv projection, attention, and the partial c_proj
contribution; the host sums the 4 tensor-parallel partials per batch
(the "all-reduce") and adds b_proj.

All matmul operands are bf16 (fp32 PSUM accumulation).  fp32-mode
(F32R) matmuls trip the PE power throttle to K=4/8 for most of the
kernel; bf16 runs the array at full clock.  Phases are software-
pipelined over 512-wide t-blocks j: causality means attention for
q-block j only needs k/v through block j, so qkv(j+1) | attn(j) |
normalize+c_proj(j-1) overlap across engines.
"""

import sys
from contextlib import ExitStack

for _p in ("/opt/trn_rl_repo",):
    if _p not in sys.path:
        sys.path.insert(0, _p)

import ml_dtypes
import numpy as np

import concourse.bass as bass
import concourse.tile as tile
from concourse import mybir
from concourse.bass_utils import run_bass_kernel_spmd

F32 = mybir.dt.float32
F32R = mybir.dt.float32r
BF16 = mybir.dt.bfloat16
EXP = mybir.ActivationFunctionType.Exp

B, T, C = 2, 2048, 1024
H, D = 16, 64          # total heads, head dim
HL = 4                 # heads per core (local)
N_CORES = 8
QB = 512               # q block width (columns of S_T)
NTT = T // 128         # 16 t-tiles
NTB = T // QB          # 4 t-blocks
NC_C = C // 128        # 8 contraction tiles over C


def _merge(a, b):
    for k, v in b.items():
        if a.get(k, -1) < v:
            a[k] = v


def _reduce_matmul_waits(nc):
    """Sound transitive reduction of Matmult sync waits.

    Walrus rejects self-loading matmuls with >1 sync wait (the LDWEIGHTS
    struct has one wait slot).  Tile emits per-proc-minimal waits but does not
    track cross-proc transitivity, so e.g. a matmul recycling a PSUM slot
    waits on both the old writer (PE) and the old reader (ACT) even though the
    reader's wait already implies the writer finished.  We compute guarantee
    vector clocks (sem -> min value) for every sem increment and drop Matmult
    waits that are implied by the instruction's queue dispatch knowledge plus
    its remaining waits.
    """
    import bass_rust
    DMA_OPS = {"InstDMACopy", "InstDMATranspose"}
    dispatch = {}    # queue -> clock known at sequencer dispatch point
    done_prev = {}   # queue -> completion clock of previous engine inst
    sem_cum = {}     # sem -> cumulative inc
    sem_hist = {}    # sem -> list[(cum, prefix-merged clock)]
    n_dropped = 0

    def clock_at(sem, v):
        for cum, snap in sem_hist.get(sem, ()):
            if cum >= v:
                return snap
        return {}

    insts = [ins for bb in nc.main_func.blocks for ins in bb.instructions]
    for ins in insts:
        si = ins.sync_info
        q = str(getattr(ins, "engine", "?"))
        opc = type(ins).__name__
        dq = dispatch.setdefault(q, {})
        waits = list(si.on_wait) if si is not None else []
        wclocks = []
        for w in waits:
            wc = dict(clock_at(w.ant_name, w.wait_value))
            if wc.get(w.ant_name, -1) < w.wait_value:
                wc[w.ant_name] = w.wait_value
            wclocks.append(wc)

        if len(waits) > 1:
            # For serially-executing engines (DVE drains its pipe per op; ACT
            # and GpSimd likewise retire in order), the previous same-queue
            # instruction has fully completed by the time this one executes,
            # so its completion clock joins the implication base.  PE overlaps
            # matmul drains, and DMA lanes are async, so they only get
            # sequencer dispatch knowledge.
            serial = opc not in DMA_OPS and not q.endswith("PE")
            keep = set(range(len(waits)))
            order = sorted(
                range(len(waits)),
                key=lambda k: 0 if not waits[k].ant_name.startswith("DMA") else 1,
            )
            for k in order:
                if len(keep) <= 1:
                    break
                base = dict(dq)
                if serial:
                    _merge(base, done_prev.get(q, {}))
                for k2 in keep:
                    if k2 != k:
                        _merge(base, wclocks[k2])
                w = waits[k]
                if base.get(w.ant_name, -1) >= w.wait_value:
                    keep.discard(k)
            if len(keep) < len(waits):
                n_dropped += len(waits) - len(keep)
                ins.sync_info = bass_rust.SyncInfo(
                    on_wait=[waits[k] for k in sorted(keep)],
                    on_update=list(si.on_update),
                )

        for wc in wclocks:
            _merge(dq, wc)

        comp = dict(dq)
        if opc not in DMA_OPS:
            _merge(comp, done_prev.get(q, {}))

        ups = list(si.on_update) if si is not None else []
        for u in ups:
            if u.update_mode != "sem-inc":
                continue
            cum = sem_cum.get(u.ant_name, 0) + u.update_value
            sem_cum[u.ant_name] = cum
            hist = sem_hist.setdefault(u.ant_name, [])
            snap = dict(hist[-1][1]) if hist else {}
            _merge(snap, comp)
            snap[u.ant_name] = cum
            hist.append((cum, snap))
        if opc not in DMA_OPS:
            comp2 = dict(comp)
            for u in ups:
                if u.update_mode == "sem-inc":
                    comp2[u.ant_name] = max(
                        comp2.get(u.ant_name, 0), sem_cum[u.ant_name])
            done_prev[q] = comp2

    bad = [
        (ins.name, [(w.ant_name, w.wait_value) for w in ins.sync_info.on_wait])
        for ins in insts
        if type(ins).__name__ == "InstMatmult"
        and ins.sync_info is not None and len(ins.sync_info.on_wait) > 1
    ]
    if bad:
        raise RuntimeError(f"{len(bad)} matmuls still have >1 wait: {bad[:8]}")

    # This walrus accepts at most ONE sync wait per instruction struct.
    # Matmuls are handled above; for everything else, hoist the extra waits
    # onto standalone single-wait Drain carriers on the same queue (the
    # sequencer executes them in order, so the semantics are unchanged).
    wid = 0
    for bb in nc.main_func.blocks:
        out_list = []
        changed = False
        for ins in bb.instructions:
            si = ins.sync_info
            if (si is not None and len(si.on_wait) > 1
                    and type(ins).__name__ != "InstMatmult"):
                waits = list(si.on_wait)
                for w in waits[:-1]:
                    d = mybir.InstDrain(name=f"WSPLIT-{wid}", ins=[], outs=[])
                    wid += 1
                    d.engine = ins.engine
                    d.sync_info = bass_rust.SyncInfo(on_wait=[w], on_update=[])
                    try:
                        nc.register_instruction(d)
                    except Exception:
                        pass
                    out_list.append(d)
                ins.sync_info = bass_rust.SyncInfo(
                    on_wait=[waits[-1]], on_update=list(si.on_update))
                changed = True
            out_list.append(ins)
        if changed:
            bb.instructions = out_list

    # This neuronxcc's walrus rejects the raw-ISA EVENT_SEMAPHORE_RANGE_CLEAR
    # Tile emits as end-of-program semaphore hygiene ("ISA wrong length").
    # It has no sync side effects and only matters for back-to-back reuse of
    # the semaphore window inside one program, so drop it.
    for bb in nc.main_func.blocks:
        kept = [i for i in bb.instructions
                if not (type(i).__name__ == "InstISA"
                        and getattr(i, "op_name", "") ==
                        "EVENT_SEMAPHORE_RANGE_CLEAR")]
        if len(kept) != len(bb.instructions):
            bb.instructions = kept


def _build_nc() -> bass.Bass:
    nc = bass.Bass()

    xt = nc.declare_dram_parameter("xt", [C, T], BF16, False)
    wqk = nc.declare_dram_parameter("wqk", [C, 512], BF16, False)
    bqk = nc.declare_dram_parameter("bqk", [1, 512], BF16, False)
    wv = nc.declare_dram_parameter("wv", [C, 256], BF16, False)
    bv = nc.declare_dram_parameter("bv", [1, 256], BF16, False)
    wp2 = nc.declare_dram_parameter("wp2", [128, 2, C], BF16, False)
    tri = nc.declare_dram_parameter("tri", [128, 128], F32, False)
    out = nc.declare_dram_parameter("out", [T, C], F32, True)

    with tile.TileContext(nc) as tc, ExitStack() as ctx:
        consts = ctx.enter_context(tc.tile_pool(name="consts", bufs=1))
        wpool = ctx.enter_context(tc.tile_pool(name="wpool", bufs=1))
        big = ctx.enter_context(tc.tile_pool(name="big", bufs=1))
        xtp = ctx.enter_context(tc.tile_pool(name="xtp", bufs=2))
        ppool = ctx.enter_context(tc.tile_pool(name="ppool", bufs=4))
        opool = ctx.enter_context(tc.tile_pool(name="opool", bufs=2))
        small = ctx.enter_context(tc.tile_pool(name="small", bufs=2))
        # PSUM budget (8 banks): ps 2x1 + s2 2x2 + psy 2x1.
        psp = ctx.enter_context(tc.tile_pool(name="psp", bufs=2, space="PSUM"))
        pss = ctx.enter_context(tc.tile_pool(name="pss", bufs=2, space="PSUM"))
        psyp = ctx.enter_context(tc.tile_pool(name="psyp", bufs=1, space="PSUM"))

        # ---- constants + first-needed weights (DMA order matters: the
        # ring drains in issue order, so qk weights + x(0) come first and
        # the first matmul doesn't wait on the v/c_proj weights) ----
        tri_sb = consts.tile([128, 128], F32)   # additive: 0 if kk<=cc else -1e30
        nc.sync.dma_start(out=tri_sb, in_=tri[:])
        ones = consts.tile([1, 512], BF16)
        warm = consts.tile([1, 8], F32)

        bqk_sb = consts.tile([1, 512], BF16)
        nc.sync.dma_start(out=bqk_sb, in_=bqk[:])
        wqk_sb = wpool.tile([128, NC_C, 512], BF16)
        nc.sync.dma_start(out=wqk_sb, in_=wqk[:].rearrange("(c p) n -> p c n", p=128))

        # ---- persistent activations ----
        qkT = big.tile([128, 4, T], BF16)    # [qk ct, t]: rows = [q01|q23|k01|k23]*64
        v_sb = big.tile([128, NTT, HL, 65], BF16)   # v natural + ones column
        # normalized y, head-pair packed: ypar[g] rows hp*64+d = head 2g+hp
        ypars = [
            big.tile([128, T], BF16, tag=f"yp{g}", name=f"yp{g}") for g in range(2)
        ]
        sums2 = big.tile([1, 16, 512], F32)   # raw denominators, col (4j+h)
        sums2r = big.tile([1, 16, 512], F32R)  # 1/denominators, DVE-written
        ones_f = consts.tile([1, 64], F32R)
        scr_y = [None]  # rolling DVE funnel of the latest psy drain

        # all constant memsets on DVE (single-proc tick, observed once by PE)
        nc.vector.memset(v_sb[:, :, :, 64:65], 1.0)
        nc.vector.memset(ones, 1.0)
        nc.vector.memset(ones_f.bitcast(F32), 1.0)
        nc.vector.memset(warm, 0.0)
        nc.scalar.activation(out=warm, in_=warm, func=EXP)  # pre-load exp table

        # Self-loading matmuls support a single sync wait, so every real
        # matmul must have at most one un-observed dependency.  Funnel all
        # input-DMA completions through DVE copies, then observe DVE once on
        # the PE via a tiny "touch" matmul.  Touches write a psp slot and
        # get a DVE reader so their slot release is a DVE-only event.
        def touch(src_ap):
            # A standalone bf16 LDWEIGHTS makes the PE observe the DVE
            # funnel without a PSUM slot (the next matmul reloads weights).
            return nc.tensor.ldweights(weights=src_ap)

        def funnel_touch(aps, k):
            scr = consts.tile([1, 8], BF16, tag=f"scr{k}", name=f"scr{k}")
            for i, t_ap in enumerate(aps):
                nc.vector.tensor_copy(out=scr[:, i:i + 1], in_=t_ap)
            scr2 = consts.tile([1, 1], BF16, tag=f"scr2{k}", name=f"scr2{k}")
            nc.vector.tensor_copy(out=scr2, in_=scr[:, 0:1])
            touch(scr2)

        funnel_touch([wqk_sb[0:1, 0, 0:1], tri_sb[0:1, 0:1], bqk_sb[:, 0:1]], 0)

        # ---------------- phase pieces ----------------

        def load_x(tb):
            ts = slice(tb * QB, (tb + 1) * QB)
            xt_tb = xtp.tile([128, NC_C, QB], BF16, tag="xt", name="xt_tb")
            nc.sync.dma_start(
                out=xt_tb, in_=xt[:, ts].rearrange("(c p) n -> p c n", p=128)
            )
            xt_t = xtp.tile([1, 1], BF16, tag="xt_t", name="xt_t")
            nc.vector.tensor_copy(out=xt_t, in_=xt_tb[0:1, 0, 0:1])
            touch(xt_t)
            return xt_tb

        def qkv_qk(tb, xt_tb):
            """q,k for t-block tb into qkT (ACT copies)."""
            ts = slice(tb * QB, (tb + 1) * QB)
            for ct in range(4):
                ps = psp.tile([128, 512], F32, tag="ps", name="ps")
                for c in range(NC_C):
                    nc.tensor.matmul(
                        out=ps,
                        lhsT=wqk_sb[:, c, ct * 128:(ct + 1) * 128],
                        rhs=xt_tb[:, c, :],
                        start=(c == 0),
                        stop=False,
                    )
                nc.tensor.matmul(  # + bias (outer product with ones row)
                    out=ps,
                    lhsT=bqk_sb[:, ct * 128:(ct + 1) * 128],
                    rhs=ones[:, 0:QB],
                    start=False,
                    stop=True,
                )
                nc.scalar.copy(out=qkT[:, ct, ts], in_=ps)

        def qkv_v(tb, xt_tb):
            """v for the 4 t-tiles of block tb into v_sb (DVE copies)."""
            for t4 in range(4):
                tt = tb * 4 + t4
                psv = psp.tile([128, 256], F32, tag="ps", name="psv")
                for c in range(NC_C):
                    nc.tensor.matmul(
                        out=psv,
                        lhsT=xt_tb[:, c, t4 * 128:(t4 + 1) * 128],
                        rhs=wv_sb[:, c, :],
                        start=(c == 0),
                        stop=False,
                    )
                nc.tensor.matmul(
                    out=psv,
                    lhsT=ones[:, 0:128],
                    rhs=bv_sb[:],
                    start=False,
                    stop=True,
                )
                nc.vector.tensor_copy(
                    out=v_sb[:, tt, :, 0:64],
                    in_=psv[:].rearrange("p (h d) -> p h d", h=HL),
                )

        def attn(j):
            """Attention for q-block j, all 4 heads.  The two heads of a
            pair write adjacent PSUM banks of one [128,2,512] tile so a
            single ACT exp covers both; S/exp run one i-tile ahead of AV
            so the exp hides under PE matmuls."""
            qs = slice(j * QB, (j + 1) * QB)
            nk = 4 * j + 4
            for g in range(2):
                psy = {}
                for hp in range(2):
                    psy[hp] = psyp.tile(
                        [65, 512], F32, tag=f"psy{hp}", name=f"psy{hp}"
                    )
                pts = None  # exp'd pair tile awaiting AV

                def s_exp(i):
                    o = max(0, 128 * i - 512 * j)
                    s2 = pss.tile([128, 2, 512], F32, tag="s2", name="s2")
                    for hp in range(2):
                        h = 2 * g + hp
                        ct_q, ct_k, base = h // 2, 2 + h // 2, (h % 2) * 64
                        rows = slice(base, base + 64)
                        nc.tensor.matmul(
                            out=s2[:, hp, o:512],
                            lhsT=qkT[rows, ct_k, i * 128:(i + 1) * 128],
                            rhs=qkT[rows, ct_q, j * QB + o:(j + 1) * QB],
                            start=True,
                            stop=True,
                            skip_group_check=True,
                        )
                        if i >= 4 * j:  # diagonal block: -inf above diagonal
                            nc.vector.tensor_add(
                                out=s2[:, hp, o:o + 128],
                                in0=s2[:, hp, o:o + 128],
                                in1=tri_sb,
                            )
                    p_t = ppool.tile([128, 2, 512], BF16, tag="p", name="p_t")
                    nc.scalar.activation(
                        out=p_t[:, :, o:512], in_=s2[:, :, o:512], func=EXP,
                        scale=0.125,
                    )
                    return (p_t, o)

                def av(i):
                    ldw = None
                    if i == 0 and scr_y[0] is not None:
                        # The psy slot handoff from the previous group would
                        # give this AV two waits (DVE drain + ACT exp);
                        # observe the previous drain on the PE first so the
                        # DVE wait is implied and dropped.  The sync=False
                        # edge pins the ldweights before the AVs in the PE
                        # stream (the scheduler would otherwise float it).
                        ldw = touch(scr_y[0])
                    p_t, o = pts
                    for hp in range(2):
                        h = 2 * g + hp
                        mm = nc.tensor.matmul(
                            out=psy[hp][:, o:512],
                            lhsT=v_sb[:, i, h, :],
                            rhs=p_t[:, hp, o:512],
                            start=(i == 0),
                            stop=(i == nk - 1),
                        )
                        if ldw is not None:
                            bass._add_dep_helper(
                                mm.ins, ldw.ins, sync=False,
                                reason="drain-observing ldweights before AV",
                            )

                for i in range(nk):
                    nxt = s_exp(i)
                    if i > 0:
                        av(i - 1)
                    pts = nxt
                av(nk - 1)

                for hp in range(2):
                    h = 2 * g + hp
                    nc.vector.tensor_copy(
                        out=sums2[:, 4 * j + h, :],
                        in_=psy[hp][64:65, :],
                    )
                    nc.vector.tensor_copy(
                        out=ypars[g][64 * hp:64 * hp + 64, qs],
                        in_=psy[hp][0:64, :],
                    )
                # sums -> 1/sums in place: exp(-ln s) on ACT (Ln and Exp
                # share the natural_log_exp table set -> no table switch).
                # Emitted per head-pair so g=0's recip runs under g=1's
                # matmuls instead of blocking the next block's exps.  The
                # DVE copy into sums2r funnels the ACT write through DVE so
                # the psb broadcast matmuls end up with a single (DVE) wait.
                s_j = sums2[:, 4 * j + 2 * g:4 * j + 2 * g + 2, :]
                nc.scalar.activation(
                    out=s_j, in_=s_j, func=mybir.ActivationFunctionType.Ln,
                )
                nc.scalar.activation(out=s_j, in_=s_j, func=EXP, scale=-1.0)
                nc.vector.tensor_copy(
                    out=sums2r[:, 4 * j + 2 * g:4 * j + 2 * g + 2, :], in_=s_j,
                )
                # trailing DVE funnel: queued after all this group's drain
                # copies, so a PE ldweights on it implies them all
                sy = consts.tile([1, 1], BF16, tag="scry", name="sy",
                                 bufs=2)
                nc.vector.tensor_copy(
                    out=sy, in_=ypars[g][0:1, j * QB:j * QB + 1]
                )
                scr_y[0] = sy

        def napply(j):
            """Broadcast 1/sums via PE outer product, scale ypar."""
            qs = slice(j * QB, (j + 1) * QB)
            for g in range(2):
                for hp in range(2):
                    h = 2 * g + hp
                    psb = psp.tile([64, 512], F32, tag="ps", name="psb")
                    nc.tensor.matmul(
                        out=psb,
                        lhsT=ones_f,
                        rhs=sums2r[:, 4 * j + h, :],
                        start=True, stop=True,
                    )
                    nc.vector.tensor_mul(
                        out=ypars[g][64 * hp:64 * hp + 64, qs],
                        in0=ypars[g][64 * hp:64 * hp + 64, qs],
                        in1=psb,
                    )

        def cproj(j):
            """c_proj partial for the 4 t-tiles of block j (head pairs give
            K=128), staged through SBUF (DVE) then DMA'd out."""
            for t4 in range(4):
                tt = 4 * j + t4
                t128 = slice(tt * 128, (tt + 1) * 128)
                ot = opool.tile([128, C], F32, tag="ot", name="ot")
                for co in range(2):
                    pso = psp.tile([128, 512], F32, tag="ps", name="pso")
                    for g in range(2):
                        nc.tensor.matmul(
                            out=pso,
                            lhsT=ypars[g][:, t128],
                            rhs=wp_sb[:, g, co * 512:(co + 1) * 512],
                            start=(g == 0),
                            stop=(g == 1),
                        )
                    nc.vector.tensor_copy(out=ot[:, co * 512:(co + 1) * 512], in_=pso)
                nc.sync.dma_start(out=out[t128, :], in_=ot)

        # ---------------- pipelined schedule ----------------
        # Causality: attn(j) needs qkv only through block j, so qkv(j+1)
        # fills the PE while ACT drains block-j copies/exps; the sums
        # reciprocal (ACT) and its PE broadcast are split so each lands
        # where the target engine has slack.
        xts = [load_x(0)]
        qkv_qk(0, xts[0])
        xts.append(load_x(1))
        # v/c_proj weights arrive behind the qk ones; gate the v matmuls
        # on their own touch so the first qk matmuls don't wait for them.
        wv_sb = wpool.tile([128, NC_C, 256], BF16)
        nc.sync.dma_start(out=wv_sb, in_=wv[:].rearrange("(c p) n -> p c n", p=128))
        bv_sb = consts.tile([1, 256], BF16)
        nc.sync.dma_start(out=bv_sb, in_=bv[:])
        funnel_touch([wv_sb[0:1, 0, 0:1], bv_sb[:, 0:1]], 1)
        qkv_v(0, xts[0])
        qkv_qk(1, xts[1])
        qkv_v(1, xts[1])
        wp_sb = wpool.tile([128, 2, C], BF16)
        nc.sync.dma_start(out=wp_sb, in_=wp2[:])
        funnel_touch([wp_sb[0:1, 0, 0:1]], 2)
        attn(0)
        xts.append(load_x(2))
        qkv_qk(2, xts[2])
        qkv_v(2, xts[2])
        napply(0)
        attn(1)
        xts.append(load_x(3))
        qkv_qk(3, xts[3])
        qkv_v(3, xts[3])
        napply(1)
        cproj(0)
        attn(2)
        napply(2)
        cproj(1)
        attn(3)
        cproj(2)
        napply(3)
        cproj(3)

    _reduce_matmul_waits(nc)
    return nc


_NC_CACHE = {}


def _get_nc():
    if "nc" not in _NC_CACHE:
        _NC_CACHE["nc"] = _build_nc()
    return _NC_CACHE["nc"]


def make_in_maps(x, W_attn, b_attn, W_proj, b_proj):
    x = np.asarray(x, dtype=np.float32)
    W_attn = np.asarray(W_attn, dtype=np.float32)
    b_attn = np.asarray(b_attn, dtype=np.float32)
    W_proj = np.asarray(W_proj, dtype=np.float32)
    kk, cc = np.meshgrid(np.arange(128), np.arange(128), indexing="ij")
    tri = np.where(kk <= cc, 0.0, -1e30).astype(np.float32)
    bf = ml_dtypes.bfloat16
    in_maps = []
    for core in range(N_CORES):
        b, hg = core // 4, core % 4
        qc = slice(hg * 256, (hg + 1) * 256)
        kc = slice(C + hg * 256, C + (hg + 1) * 256)
        vc = slice(2 * C + hg * 256, 2 * C + (hg + 1) * 256)
        # W_proj rows for this head group, packed by head pair:
        # wp2[hp*64+d, g, :] = W_proj[hg*256 + (2g+hp)*64 + d, :]
        wp = W_proj[hg * 256:(hg + 1) * 256].reshape(2, 2, 64, C)  # [g,hp,d,C]
        wp2 = np.ascontiguousarray(wp.transpose(1, 2, 0, 3).reshape(128, 2, C))
        in_maps.append({
            "xt": np.ascontiguousarray(x[b].T).astype(bf),
            "wqk": np.ascontiguousarray(
                np.concatenate([W_attn[:, qc], W_attn[:, kc]], axis=1)).astype(bf),
            "bqk": np.concatenate([b_attn[qc], b_attn[kc]]).reshape(1, 512).astype(bf),
            "wv": np.ascontiguousarray(W_attn[:, vc]).astype(bf),
            "bv": b_attn[vc].reshape(1, 256).astype(bf),
            "wp2": wp2.astype(bf),
            "tri": tri,
        })
    return in_maps


def gather(results, b_proj):
    b_proj = np.asarray(b_proj, dtype=np.float32)
    out = np.empty((B, T, C), dtype=np.float32)
    for b in range(B):
        acc = results[4 * b]["out"].astype(np.float32)
        for g in range(1, 4):
            acc = acc + results[4 * b + g]["out"]
        out[b] = acc + b_proj
    return out


def run(x, W_attn, b_attn, W_proj, b_proj, trace=False):
    nc = _get_nc()
    in_maps = make_in_maps(x, W_attn, b_attn, W_proj, b_proj)
    res = run_bass_kernel_spmd(nc, in_maps, list(range(N_CORES)), trace=trace)
    return gather(res.results, b_proj), res


def kernel(x, W_attn, b_attn, W_proj, b_proj):
    out, _ = run(x, W_attn, b_attn, W_proj, b_proj)
    return out


# revision 47
# speedup vs baseline: 1.3208x; 1.0618x over previous
"""Causal self-attention (B=2, T=2048, C=1024, H=16) on 8 trn2 NeuronCores.

Sharding: data-parallel over B (2) x tensor-parallel over head groups (4
groups of 4 heads).  core c -> batch c//4, head group c%4.  Each core
computes its 4 heads' qkv projection, attention, and the partial c_proj
contribution; the host sums the 4 tensor-parallel partials per batch
(the "all-reduce") and adds b_proj.

All matmul operands are bf16 (fp32 PSUM accumulation).  fp32-mode
(F32R) matmuls trip the PE power throttle to K=4/8 for most of the
kernel; bf16 runs the array at full clock.  Phases are software-
pipelined over 512-wide t-blocks j: causality means attention for
q-block j only needs k/v through block j, so qkv(j+1) | attn(j) |
normalize+c_proj(j-1) overlap across engines.
"""

import sys
from contextlib import ExitStack

for _p in ("/opt/trn_rl_repo",):
    if _p not in sys.path:
        sys.path.insert(0, _p)

import ml_dtypes
import numpy as np

import concourse.bass as bass
import concourse.tile as tile
from concourse import mybir
from concourse.bass_utils import run_bass_kernel_spmd

F32 = mybir.dt.float32
F32R = mybir.dt.float32r
BF16 = mybir.dt.bfloat16
EXP = mybir.ActivationFunctionType.Exp

B, T, C = 2, 2048, 1024
H, D = 16, 64          # total heads, head dim
HL = 4                 # heads per core (local)
N_CORES = 8
QB = 512               # q block width (columns of S_T)
NTT = T // 128         # 16 t-tiles
NTB = T // QB          # 4 t-blocks
NC_C = C // 128        # 8 contraction tiles over C


def _merge(a, b):
    for k, v in b.items():
        if a.get(k, -1) < v:
            a[k] = v


def _reduce_matmul_waits(nc):
    """Sound transitive reduction of Matmult sync waits.

    Walrus rejects self-loading matmuls with >1 sync wait (the LDWEIGHTS
    struct has one wait slot).  Tile emits per-proc-minimal waits but does not
    track cross-proc transitivity, so e.g. a matmul recycling a PSUM slot
    waits on both the old writer (PE) and the old reader (ACT) even though the
    reader's wait already implies the writer finished.  We compute guarantee
    vector clocks (sem -> min value) for every sem increment and drop Matmult
    waits that are implied by the instruction's queue dispatch knowledge plus
    its remaining waits.
    """
    import bass_rust
    DMA_OPS = {"InstDMACopy", "InstDMATranspose"}
    dispatch = {}    # queue -> clock known at sequencer dispatch point
    done_prev = {}   # queue -> completion clock of previous engine inst
    sem_cum = {}     # sem -> cumulative inc
    sem_hist = {}    # sem -> list[(cum, prefix-merged clock)]
    n_dropped = 0

    def clock_at(sem, v):
        for cum, snap in sem_hist.get(sem, ()):
            if cum >= v:
                return snap
        return {}

    insts = [ins for bb in nc.main_func.blocks for ins in bb.instructions]
    for ins in insts:
        si = ins.sync_info
        q = str(getattr(ins, "engine", "?"))
        opc = type(ins).__name__
        dq = dispatch.setdefault(q, {})
        waits = list(si.on_wait) if si is not None else []
        wclocks = []
        for w in waits:
            wc = dict(clock_at(w.ant_name, w.wait_value))
            if wc.get(w.ant_name, -1) < w.wait_value:
                wc[w.ant_name] = w.wait_value
            wclocks.append(wc)

        if len(waits) > 1:
            # For serially-executing engines (DVE drains its pipe per op; ACT
            # and GpSimd likewise retire in order), the previous same-queue
            # instruction has fully completed by the time this one executes,
            # so its completion clock joins the implication base.  PE overlaps
            # matmul drains, and DMA lanes are async, so they only get
            # sequencer dispatch knowledge.
            serial = opc not in DMA_OPS and not q.endswith("PE")
            keep = set(range(len(waits)))
            order = sorted(
                range(len(waits)),
                key=lambda k: 0 if not waits[k].ant_name.startswith("DMA") else 1,
            )
            for k in order:
                if len(keep) <= 1:
                    break
                base = dict(dq)
                if serial:
                    _merge(base, done_prev.get(q, {}))
                for k2 in keep:
                    if k2 != k:
                        _merge(base, wclocks[k2])
                w = waits[k]
                if base.get(w.ant_name, -1) >= w.wait_value:
                    keep.discard(k)
            if len(keep) < len(waits):
                n_dropped += len(waits) - len(keep)
                ins.sync_info = bass_rust.SyncInfo(
                    on_wait=[waits[k] for k in sorted(keep)],
                    on_update=list(si.on_update),
                )

        for wc in wclocks:
            _merge(dq, wc)

        comp = dict(dq)
        if opc not in DMA_OPS:
            _merge(comp, done_prev.get(q, {}))

        ups = list(si.on_update) if si is not None else []
        for u in ups:
            if u.update_mode != "sem-inc":
                continue
            cum = sem_cum.get(u.ant_name, 0) + u.update_value
            sem_cum[u.ant_name] = cum
            hist = sem_hist.setdefault(u.ant_name, [])
            snap = dict(hist[-1][1]) if hist else {}
            _merge(snap, comp)
            snap[u.ant_name] = cum
            hist.append((cum, snap))
        if opc not in DMA_OPS:
            comp2 = dict(comp)
            for u in ups:
                if u.update_mode == "sem-inc":
                    comp2[u.ant_name] = max(
                        comp2.get(u.ant_name, 0), sem_cum[u.ant_name])
            done_prev[q] = comp2

    bad = [
        (ins.name, [(w.ant_name, w.wait_value) for w in ins.sync_info.on_wait])
        for ins in insts
        if type(ins).__name__ == "InstMatmult"
        and ins.sync_info is not None and len(ins.sync_info.on_wait) > 1
    ]
    if bad:
        raise RuntimeError(f"{len(bad)} matmuls still have >1 wait: {bad[:8]}")

    # This walrus accepts at most ONE sync wait per instruction struct.
    # Matmuls are handled above; for everything else, hoist the extra waits
    # onto standalone single-wait Drain carriers on the same queue (the
    # sequencer executes them in order, so the semantics are unchanged).
    wid = 0
    for bb in nc.main_func.blocks:
        out_list = []
        changed = False
        for ins in bb.instructions:
            si = ins.sync_info
            if (si is not None and len(si.on_wait) > 1
                    and type(ins).__name__ != "InstMatmult"):
                waits = list(si.on_wait)
                for w in waits[:-1]:
                    d = mybir.InstDrain(name=f"WSPLIT-{wid}", ins=[], outs=[])
                    wid += 1
                    d.engine = ins.engine
                    d.sync_info = bass_rust.SyncInfo(on_wait=[w], on_update=[])
                    try:
                        nc.register_instruction(d)
                    except Exception:
                        pass
                    out_list.append(d)
                ins.sync_info = bass_rust.SyncInfo(
                    on_wait=[waits[-1]], on_update=list(si.on_update))
                changed = True
            out_list.append(ins)
        if changed:
            bb.instructions = out_list

    # This neuronxcc's walrus rejects the raw-ISA EVENT_SEMAPHORE_RANGE_CLEAR
    # Tile emits as end-of-program semaphore hygiene ("ISA wrong length").
    # It has no sync side effects and only matters for back-to-back reuse of
    # the semaphore window inside one program, so drop it.
    for bb in nc.main_func.blocks:
        kept = [i for i in bb.instructions
                if not (type(i).__name__ == "InstISA"
                        and getattr(i, "op_name", "") ==
                        "EVENT_SEMAPHORE_RANGE_CLEAR")]
        if len(kept) != len(bb.instructions):
            bb.instructions = kept


def _build_nc() -> bass.Bass:
    nc = bass.Bass()

    # host pre-layouts make every DMA line contiguous per partition
    xt4 = nc.declare_dram_parameter("xt4", [NTB, 128, NC_C, QB], BF16, False)
    wqk = nc.declare_dram_parameter("wqk", [128, NC_C, 512], BF16, False)
    bqk = nc.declare_dram_parameter("bqk", [1, 512], BF16, False)
    wv = nc.declare_dram_parameter("wv", [128, NC_C, 256], BF16, False)
    bv = nc.declare_dram_parameter("bv", [1, 256], BF16, False)
    wp2 = nc.declare_dram_parameter("wp2", [128, 2, C], BF16, False)
    trib = nc.declare_dram_parameter("trib", [128, 128], BF16, False)
    idn = nc.declare_dram_parameter("idn", [128, 128], BF16, False)
    out = nc.declare_dram_parameter("out", [T, C], BF16, True)

    with tile.TileContext(nc) as tc, ExitStack() as ctx:
        consts = ctx.enter_context(tc.tile_pool(name="consts", bufs=1))
        wpool = ctx.enter_context(tc.tile_pool(name="wpool", bufs=1))
        big = ctx.enter_context(tc.tile_pool(name="big", bufs=1))
        xtp = ctx.enter_context(tc.tile_pool(name="xtp", bufs=2))
        ppool = ctx.enter_context(tc.tile_pool(name="ppool", bufs=4))
        opool = ctx.enter_context(tc.tile_pool(name="opool", bufs=2))
        small = ctx.enter_context(tc.tile_pool(name="small", bufs=2))
        # PSUM budget (8 banks): ps 2x1 + s2 2x2 + psy 2x1.
        psp = ctx.enter_context(tc.tile_pool(name="psp", bufs=2, space="PSUM"))
        pss = ctx.enter_context(tc.tile_pool(name="pss", bufs=2, space="PSUM"))
        psyp = ctx.enter_context(tc.tile_pool(name="psyp", bufs=1, space="PSUM"))

        # ---- constants + first-needed weights (DMA order matters: the
        # ring drains in issue order, so qk weights + x(0) come first and
        # the first matmul doesn't wait on the v/c_proj weights) ----
        tri_sb = consts.tile([128, 128], BF16)  # M^T: mask via PE outer sum
        nc.sync.dma_start(out=tri_sb, in_=trib[:])
        idn_sb = consts.tile([128, 128], BF16)
        nc.sync.dma_start(out=idn_sb, in_=idn[:])
        ones = consts.tile([1, 512], BF16)
        warm = consts.tile([1, 8], F32)

        bqk_sb = consts.tile([1, 512], BF16)
        nc.sync.dma_start(out=bqk_sb, in_=bqk[:])
        wqk_sb = wpool.tile([128, NC_C, 512], BF16)
        nc.sync.dma_start(out=wqk_sb, in_=wqk[:])

        # ---- persistent activations ----
        qkT = big.tile([128, 4, T], BF16)    # [qk ct, t]: rows = [q01|q23|k01|k23]*64
        v_sb = big.tile([128, NTT, HL, 65], BF16)   # v natural + ones column
        # normalized y, head-pair packed: ypar[g] rows hp*64+d = head 2g+hp
        ypars = [
            big.tile([128, T], BF16, tag=f"yp{g}", name=f"yp{g}") for g in range(2)
        ]
        sums2 = big.tile([1, 16, 512], F32)   # raw denominators, col (4j+h)
        sums2r = big.tile([1, 16, 512], F32R)  # 1/denominators, DVE-written
        ones_f = consts.tile([1, 64], F32R)
        scr_y = [None]  # rolling DVE funnel of the latest psy drain

        # all constant memsets on DVE (single-proc tick, observed once by PE)
        nc.vector.memset(v_sb[:, :, :, 64:65], 1.0)
        nc.vector.memset(ones, 1.0)
        nc.vector.memset(ones_f.bitcast(F32), 1.0)
        nc.vector.memset(warm, 0.0)
        nc.scalar.activation(out=warm, in_=warm, func=EXP)  # pre-load exp table

        # Self-loading matmuls support a single sync wait, so every real
        # matmul must have at most one un-observed dependency.  Funnel all
        # input-DMA completions through DVE copies, then observe DVE once on
        # the PE via a tiny "touch" matmul.  Touches write a psp slot and
        # get a DVE reader so their slot release is a DVE-only event.
        def touch(src_ap):
            # A standalone bf16 LDWEIGHTS makes the PE observe the DVE
            # funnel without a PSUM slot (the next matmul reloads weights).
            return nc.tensor.ldweights(weights=src_ap)

        def funnel_touch(aps, k):
            scr = consts.tile([1, 8], BF16, tag=f"scr{k}", name=f"scr{k}")
            for i, t_ap in enumerate(aps):
                nc.vector.tensor_copy(out=scr[:, i:i + 1], in_=t_ap)
            scr2 = consts.tile([1, 1], BF16, tag=f"scr2{k}", name=f"scr2{k}")
            nc.vector.tensor_copy(out=scr2, in_=scr[:, 0:1])
            touch(scr2)

        funnel_touch([wqk_sb[0:1, 0, 0:1], tri_sb[0:1, 0:1],
                      idn_sb[0:1, 0:1], bqk_sb[:, 0:1]], 0)

        # ---------------- phase pieces ----------------

        def load_x(tb):
            xt_tb = xtp.tile([128, NC_C, QB], BF16, tag="xt", name="xt_tb")
            nc.sync.dma_start(out=xt_tb, in_=xt4[tb])
            xt_t = xtp.tile([1, 1], BF16, tag="xt_t", name="xt_t")
            nc.vector.tensor_copy(out=xt_t, in_=xt_tb[0:1, 0, 0:1])
            touch(xt_t)
            return xt_tb

        def qkv_qk(tb, xt_tb):
            """q,k for t-block tb into qkT (ACT copies)."""
            ts = slice(tb * QB, (tb + 1) * QB)
            for ct in range(4):
                ps = psp.tile([128, 512], F32, tag="ps", name="ps")
                for c in range(NC_C):
                    nc.tensor.matmul(
                        out=ps,
                        lhsT=wqk_sb[:, c, ct * 128:(ct + 1) * 128],
                        rhs=xt_tb[:, c, :],
                        start=(c == 0),
                        stop=False,
                    )
                nc.tensor.matmul(  # + bias (outer product with ones row)
                    out=ps,
                    lhsT=bqk_sb[:, ct * 128:(ct + 1) * 128],
                    rhs=ones[:, 0:QB],
                    start=False,
                    stop=True,
                )
                nc.scalar.copy(out=qkT[:, ct, ts], in_=ps)

        def qkv_v(tb, xt_tb):
            """v for the 4 t-tiles of block tb into v_sb (DVE copies)."""
            for t4 in range(4):
                tt = tb * 4 + t4
                psv = psp.tile([128, 256], F32, tag="ps", name="psv")
                for c in range(NC_C):
                    nc.tensor.matmul(
                        out=psv,
                        lhsT=xt_tb[:, c, t4 * 128:(t4 + 1) * 128],
                        rhs=wv_sb[:, c, :],
                        start=(c == 0),
                        stop=False,
                    )
                nc.tensor.matmul(
                    out=psv,
                    lhsT=ones[:, 0:128],
                    rhs=bv_sb[:],
                    start=False,
                    stop=True,
                )
                nc.vector.tensor_copy(
                    out=v_sb[:, tt, :, 0:64],
                    in_=psv[:].rearrange("p (h d) -> p h d", h=HL),
                )

        def attn(j):
            """Attention for q-block j, all 4 heads.  The two heads of a
            pair write adjacent PSUM banks of one [128,2,512] tile so a
            single ACT exp covers both; S/exp run one i-tile ahead of AV
            so the exp hides under PE matmuls."""
            qs = slice(j * QB, (j + 1) * QB)
            nk = 4 * j + 4
            for g in range(2):
                psy = {}
                for hp in range(2):
                    psy[hp] = psyp.tile(
                        [65, 512], F32, tag=f"psy{hp}", name=f"psy{hp}"
                    )
                pts = None  # exp'd pair tile awaiting AV

                def s_exp(i):
                    o = max(0, 128 * i - 512 * j)
                    diag = i >= 4 * j
                    s2 = pss.tile([128, 2, 512], F32, tag="s2", name="s2")
                    for hp in range(2):
                        h = 2 * g + hp
                        ct_q, ct_k, base = h // 2, 2 + h // 2, (h % 2) * 64
                        rows = slice(base, base + 64)
                        if diag:
                            # diagonal block: seed the 128-wide strip with
                            # the -inf mask on the PE (tri^T @ I), then let
                            # the S matmul accumulate on top -- keeps the
                            # mask off the congested DVE queue
                            nc.tensor.matmul(
                                out=s2[:, hp, o:o + 128],
                                lhsT=tri_sb,
                                rhs=idn_sb,
                                start=True,
                                stop=False,
                                skip_group_check=True,
                            )
                        nc.tensor.matmul(
                            out=s2[:, hp, o:512],
                            lhsT=qkT[rows, ct_k, i * 128:(i + 1) * 128],
                            rhs=qkT[rows, ct_q, j * QB + o:(j + 1) * QB],
                            start=not diag,
                            stop=True,
                            skip_group_check=True,
                        )
                    p_t = ppool.tile([128, 2, 512], BF16, tag="p", name="p_t")
                    nc.scalar.activation(
                        out=p_t[:, :, o:512], in_=s2[:, :, o:512], func=EXP,
                        scale=0.125,
                    )
                    return (p_t, o)

                def av(i):
                    ldw = None
                    if i == 0 and scr_y[0] is not None:
                        # The psy slot handoff from the previous group would
                        # give this AV two waits (DVE drain + ACT exp);
                        # observe the previous drain on the PE first so the
                        # DVE wait is implied and dropped.  The sync=False
                        # edge pins the ldweights before the AVs in the PE
                        # stream (the scheduler would otherwise float it).
                        ldw = touch(scr_y[0])
                    p_t, o = pts
                    for hp in range(2):
                        h = 2 * g + hp
                        mm = nc.tensor.matmul(
                            out=psy[hp][:, o:512],
                            lhsT=v_sb[:, i, h, :],
                            rhs=p_t[:, hp, o:512],
                            start=(i == 0),
                            stop=(i == nk - 1),
                        )
                        if ldw is not None:
                            bass._add_dep_helper(
                                mm.ins, ldw.ins, sync=False,
                                reason="drain-observing ldweights before AV",
                            )

                for i in range(nk):
                    nxt = s_exp(i)
                    if i > 0:
                        av(i - 1)
                    pts = nxt
                av(nk - 1)

                for hp in range(2):
                    h = 2 * g + hp
                    nc.vector.tensor_copy(
                        out=sums2[:, 4 * j + h, :],
                        in_=psy[hp][64:65, :],
                    )
                    nc.vector.tensor_copy(
                        out=ypars[g][64 * hp:64 * hp + 64, qs],
                        in_=psy[hp][0:64, :],
                    )
                # sums -> 1/sums in place: exp(-ln s) on ACT (Ln and Exp
                # share the natural_log_exp table set -> no table switch).
                # Emitted per head-pair so g=0's recip runs under g=1's
                # matmuls instead of blocking the next block's exps.  The
                # DVE copy into sums2r funnels the ACT write through DVE so
                # the psb broadcast matmuls end up with a single (DVE) wait.
                s_j = sums2[:, 4 * j + 2 * g:4 * j + 2 * g + 2, :]
                nc.scalar.activation(
                    out=s_j, in_=s_j, func=mybir.ActivationFunctionType.Ln,
                )
                nc.scalar.activation(out=s_j, in_=s_j, func=EXP, scale=-1.0)
                nc.vector.tensor_copy(
                    out=sums2r[:, 4 * j + 2 * g:4 * j + 2 * g + 2, :], in_=s_j,
                )
                # trailing DVE funnel: queued after all this group's drain
                # copies, so a PE ldweights on it implies them all
                sy = consts.tile([1, 1], BF16, tag="scry", name="sy",
                                 bufs=2)
                nc.vector.tensor_copy(
                    out=sy, in_=ypars[g][0:1, j * QB:j * QB + 1]
                )
                scr_y[0] = sy

        def napply(j):
            """Broadcast 1/sums via PE outer product, scale ypar."""
            qs = slice(j * QB, (j + 1) * QB)
            for g in range(2):
                for hp in range(2):
                    h = 2 * g + hp
                    psb = psp.tile([64, 512], F32, tag="ps", name="psb")
                    nc.tensor.matmul(
                        out=psb,
                        lhsT=ones_f,
                        rhs=sums2r[:, 4 * j + h, :],
                        start=True, stop=True,
                    )
                    nc.vector.tensor_mul(
                        out=ypars[g][64 * hp:64 * hp + 64, qs],
                        in0=ypars[g][64 * hp:64 * hp + 64, qs],
                        in1=psb,
                    )

        def cproj(j):
            """c_proj partial for the 4 t-tiles of block j (head pairs give
            K=128), staged through SBUF (DVE) then DMA'd out."""
            for t4 in range(4):
                tt = 4 * j + t4
                t128 = slice(tt * 128, (tt + 1) * 128)
                ot = opool.tile([128, C], BF16, tag="ot", name="ot")
                for co in range(2):
                    pso = psp.tile([128, 512], F32, tag="ps", name="pso")
                    for g in range(2):
                        nc.tensor.matmul(
                            out=pso,
                            lhsT=ypars[g][:, t128],
                            rhs=wp_sb[:, g, co * 512:(co + 1) * 512],
                            start=(g == 0),
                            stop=(g == 1),
                        )
                    nc.vector.tensor_copy(out=ot[:, co * 512:(co + 1) * 512], in_=pso)
                nc.sync.dma_start(out=out[t128, :], in_=ot)

        # ---------------- pipelined schedule ----------------
        # Causality: attn(j) needs qkv only through block j, so qkv(j+1)
        # fills the PE while ACT drains block-j copies/exps; the sums
        # reciprocal (ACT) and its PE broadcast are split so each lands
        # where the target engine has slack.
        xts = [load_x(0)]
        qkv_qk(0, xts[0])
        xts.append(load_x(1))
        # v/c_proj weights arrive behind the qk ones; gate the v matmuls
        # on their own touch so the first qk matmuls don't wait for them.
        wv_sb = wpool.tile([128, NC_C, 256], BF16)
        nc.sync.dma_start(out=wv_sb, in_=wv[:])
        bv_sb = consts.tile([1, 256], BF16)
        nc.sync.dma_start(out=bv_sb, in_=bv[:])
        funnel_touch([wv_sb[0:1, 0, 0:1], bv_sb[:, 0:1]], 1)
        qkv_v(0, xts[0])
        qkv_qk(1, xts[1])
        qkv_v(1, xts[1])
        wp_sb = wpool.tile([128, 2, C], BF16)
        nc.sync.dma_start(out=wp_sb, in_=wp2[:])
        funnel_touch([wp_sb[0:1, 0, 0:1]], 2)
        attn(0)
        xts.append(load_x(2))
        qkv_qk(2, xts[2])
        qkv_v(2, xts[2])
        napply(0)
        attn(1)
        xts.append(load_x(3))
        qkv_qk(3, xts[3])
        qkv_v(3, xts[3])
        napply(1)
        cproj(0)
        attn(2)
        napply(2)
        cproj(1)
        attn(3)
        cproj(2)
        napply(3)
        cproj(3)

    _reduce_matmul_waits(nc)
    return nc


_NC_CACHE = {}


def _get_nc():
    if "nc" not in _NC_CACHE:
        _NC_CACHE["nc"] = _build_nc()
    return _NC_CACHE["nc"]


def make_in_maps(x, W_attn, b_attn, W_proj, b_proj):
    x = np.asarray(x, dtype=np.float32)
    W_attn = np.asarray(W_attn, dtype=np.float32)
    b_attn = np.asarray(b_attn, dtype=np.float32)
    W_proj = np.asarray(W_proj, dtype=np.float32)
    kk, cc = np.meshgrid(np.arange(128), np.arange(128), indexing="ij")
    tri = np.where(kk <= cc, 0.0, -1e30).astype(np.float32)
    bf = ml_dtypes.bfloat16
    trib = np.ascontiguousarray(tri.T).astype(bf)   # M^T for the PE mask mm
    idn = np.eye(128, dtype=np.float32).astype(bf)

    def p_major(w, inner):  # [C, n] -> [128, C//128, n] (partition-contiguous)
        return np.ascontiguousarray(
            w.reshape(NC_C, 128, inner).transpose(1, 0, 2)).astype(bf)

    in_maps = []
    for core in range(N_CORES):
        b, hg = core // 4, core % 4
        qc = slice(hg * 256, (hg + 1) * 256)
        kc = slice(C + hg * 256, C + (hg + 1) * 256)
        vc = slice(2 * C + hg * 256, 2 * C + (hg + 1) * 256)
        # W_proj rows for this head group, packed by head pair:
        # wp2[hp*64+d, g, :] = W_proj[hg*256 + (2g+hp)*64 + d, :]
        wp = W_proj[hg * 256:(hg + 1) * 256].reshape(2, 2, 64, C)  # [g,hp,d,C]
        wp2 = np.ascontiguousarray(wp.transpose(1, 2, 0, 3).reshape(128, 2, C))
        xt = x[b].T  # [C, T]
        xt4 = np.ascontiguousarray(
            xt.reshape(NC_C, 128, NTB, QB).transpose(2, 1, 0, 3)).astype(bf)
        in_maps.append({
            "xt4": xt4,
            "wqk": p_major(
                np.concatenate([W_attn[:, qc], W_attn[:, kc]], axis=1), 512),
            "bqk": np.concatenate([b_attn[qc], b_attn[kc]]).reshape(1, 512).astype(bf),
            "wv": p_major(W_attn[:, vc], 256),
            "bv": b_attn[vc].reshape(1, 256).astype(bf),
            "wp2": wp2.astype(bf),
            "trib": trib,
            "idn": idn,
        })
    return in_maps


def gather(results, b_proj):
    b_proj = np.asarray(b_proj, dtype=np.float32)
    out = np.empty((B, T, C), dtype=np.float32)
    for b in range(B):
        acc = results[4 * b]["out"].astype(np.float32)
        for g in range(1, 4):
            acc = acc + results[4 * b + g]["out"].astype(np.float32)
        out[b] = acc + b_proj
    return out


def run(x, W_attn, b_attn, W_proj, b_proj, trace=False):
    nc = _get_nc()
    in_maps = make_in_maps(x, W_attn, b_attn, W_proj, b_proj)
    res = run_bass_kernel_spmd(nc, in_maps, list(range(N_CORES)), trace=trace)
    return gather(res.results, b_proj), res


def kernel(x, W_attn, b_attn, W_proj, b_proj):
    out, _ = run(x, W_attn, b_attn, W_proj, b_proj)
    return out
